# revision 2
# baseline (speedup 1.0000x reference)
"""Trainium2 Bass kernel for 16-head causal MultiHeadAttention (S=4096, E=1024).

Sharding: tensor-parallel over heads across 8 NeuronCores, with
sequence-sharded I/O to minimize host<->device traffic (the wall-clock
bottleneck on tunneled cores):

- input: each core receives only its 512-column block of hT ([E, 512], 2 MiB)
  plus its per-head weight slices; an on-device AllGather reconstructs the
  full hT in local DRAM.
- each core computes QKV projection for its 2 heads, flash-style causal
  attention in scoresT layout ([t, s_q], softmax denominator via a
  ones-column appended to V so no partition reductions are needed), and a
  partial out-projection over its 128 ctx channels into a DRAM scratch.
- output: an on-device ReduceScatter(add) sums the 8 partial [S, E] outputs
  and hands each core its 512-row slice, so only 2 MiB/core returns to the
  host. The host concatenates slices and adds out_b.

Matmuls run as float32r (TF32-like, 1 cycle/row) with fp32 PSUM accumulation.
QKV projection, attention, and out-projection are fused into one loop over
512-row s-blocks (causality makes block j's attention depend only on K/V from
blocks <= j), so TensorE/ACT/DVE/DMA overlap across stages. The causal
boundary mask is generated on device with affine_select.
"""

import numpy as np

import concourse.bass as bass
import concourse.bacc as bacc
import concourse.mybir as mybir
from concourse.bass_utils import run_bass_kernel_spmd
from concourse.masks import make_identity
from concourse.tile import TileContext

N_CORES = 8
S = 4096
E = 1024
H = 16
D = 64
HPC = H // N_CORES          # heads per core = 2
C = HPC * D                 # ctx channels per core = 128
SCALE = 1.0 / np.sqrt(np.float32(E))  # note: sqrt(n_embd), per reference

SB = 512                    # s_q block (matmul free dim)
NSB = S // SB               # 8
TB = 128                    # t chunk (matmul contraction tile)
EB = 128                    # e chunk of the hidden dim
NEB = E // EB               # 8

F32 = mybir.dt.float32
F32R = mybir.dt.float32r

_COMPILED = None
last_results = None  # test harness reads exec_time_ns off this


def _build():
    nc = bacc.Bacc(None, target_bir_lowering=False, num_devices=N_CORES)

    hTs = nc.declare_dram_parameter("hTs", [E, SB], F32R, isOutput=False)
    wq = nc.declare_dram_parameter("wq", [E, C], F32R, isOutput=False)
    wk = nc.declare_dram_parameter("wk", [E, C], F32R, isOutput=False)
    wv = nc.declare_dram_parameter("wv", [E, C], F32R, isOutput=False)
    bq = nc.declare_dram_parameter("bq", [1, C], F32R, isOutput=False)
    bk = nc.declare_dram_parameter("bk", [1, C], F32R, isOutput=False)
    bv = nc.declare_dram_parameter("bv", [1, C], F32R, isOutput=False)
    wo = nc.declare_dram_parameter("wo", [C, E], F32R, isOutput=False)
    ys = nc.declare_dram_parameter("ys", [SB, E], F32, isOutput=True)

    groups = [list(range(N_CORES))]

    with TileContext(nc) as tc:
        with (
            tc.tile_pool(name="dram", bufs=1, space="DRAM") as dram,
            tc.tile_pool(name="singles", bufs=1) as singles,
            tc.tile_pool(name="big", bufs=1) as big,
            tc.tile_pool(name="htp", bufs=18) as htp,
            tc.tile_pool(name="vtf", bufs=3) as vtf,
            tc.tile_pool(name="ep", bufs=8) as ep,
            tc.tile_pool(name="ef", bufs=3) as ef,
            tc.tile_pool(name="ip", bufs=3) as ip,
            tc.tile_pool(name="pqkv", bufs=1, space="PSUM") as pqkv,
            tc.tile_pool(name="pmix", bufs=1, space="PSUM") as pmix,
            tc.tile_pool(name="psc", bufs=3, space="PSUM") as psc,
            tc.tile_pool(name="pctx", bufs=1, space="PSUM") as pctx,
            tc.tile_pool(name="yp", bufs=4) as yp,
        ):
            # ---- gather the sequence-sharded hidden states on device ----
            hTs_b = dram.tile([E, SB], F32R)
            nc.sync.dma_start(out=hTs_b[:], in_=hTs[:])
            hT_g = dram.tile([NSB, E, SB], F32R, addr_space="Shared")
            nc.gpsimd.collective_compute(
                "AllGather",
                mybir.AluOpType.bypass,
                replica_groups=groups,
                ins=[hTs_b[:].opt()],
                outs=[hT_g[:].opt()],
            )
            y_part = dram.tile([S, E], F32)   # this core's partial out-proj

            # Weights, biases, constants
            wq_sb = singles.tile([EB, NEB, C], F32R)
            wk_sb = singles.tile([EB, NEB, C], F32R)
            wv_sb = singles.tile([EB, NEB, C], F32R)
            for w_dram, w_sb in ((wq, wq_sb), (wk, wk_sb), (wv, wv_sb)):
                nc.sync.dma_start(
                    out=w_sb[:], in_=w_dram.rearrange("(a p) m -> p a m", p=EB)
                )
            wo_sb = singles.tile([C, E], F32R)
            nc.sync.dma_start(out=wo_sb[:], in_=wo[:])
            bq_sb = singles.tile([1, C], F32R)
            bk_sb = singles.tile([1, C], F32R)
            bv_sb = singles.tile([1, C], F32R)
            nc.sync.dma_start(out=bq_sb[:], in_=bq[:])
            nc.sync.dma_start(out=bk_sb[:], in_=bk[:])
            nc.sync.dma_start(out=bv_sb[:], in_=bv[:])

            # causal step mask: mask0[p, f] = 1.0 where p <= f else 0.0
            mask0 = singles.tile([TB, SB], F32)
            nc.gpsimd.memset(mask0[:], 1.0)
            nc.gpsimd.affine_select(
                out=mask0[:], in_=mask0[:],
                compare_op=mybir.AluOpType.is_ge,
                fill=0.0,
                base=0,
                pattern=[[1, SB]],       # +1 per free element f
                channel_multiplier=-1,   # -1 per partition p => f - p >= 0
            )

            ones_f = singles.tile([1, SB], F32)
            nc.vector.memset(ones_f[:], 1.0)
            ones_r = singles.tile([1, SB], F32R)
            nc.vector.tensor_copy(ones_r[:], ones_f[:])
            ones_col_f = singles.tile([TB, 1], F32)
            nc.vector.memset(ones_col_f[:], 1.0)
            ident = singles.tile([TB, TB], F32)
            make_identity(nc, ident[:])

            # Persistent activations
            qT_sb = big.tile([C, S], F32R)      # [c, s]
            kT_sb = big.tile([C, S], F32R)
            v_sb = big.tile([TB, S // TB, 2 * (D + 1)], F32R)
            ctxT_sb = big.tile([C, S], F32R)

            ones64_f = singles.tile([1, D], F32)
            nc.vector.memset(ones64_f[:], 1.0)
            ones64_r = singles.tile([1, D], F32R)
            nc.vector.tensor_copy(ones64_r[:], ones64_f[:])

            for j in range(NSB):
                # ---- QKV projection for s-block j: three sequential
                # single-bank passes (q, k, v) over the held hT tiles ----
                hts = []
                for i in range(NEB):
                    ht = htp.tile([EB, SB], F32R)
                    hts.append(ht)
                    nc.sync.dma_start(
                        out=ht[:], in_=hT_g[j, i * EB:(i + 1) * EB, :]
                    )
                ps_q = pqkv.tile([C, SB], F32, tag="q")
                for i in range(NEB):
                    nc.tensor.matmul(
                        ps_q[:], wq_sb[:, i, :], hts[i][:], start=(i == 0), stop=False
                    )
                nc.tensor.matmul(ps_q[:], bq_sb[:], ones_r[:], start=False, stop=True)
                nc.vector.tensor_copy(qT_sb[:, j * SB:(j + 1) * SB], ps_q[:])
                ps_k = pqkv.tile([C, SB], F32, tag="q")
                for i in range(NEB):
                    nc.tensor.matmul(
                        ps_k[:], wk_sb[:, i, :], hts[i][:], start=(i == 0), stop=False
                    )
                nc.tensor.matmul(ps_k[:], bk_sb[:], ones_r[:], start=False, stop=True)
                nc.vector.tensor_copy(kT_sb[:, j * SB:(j + 1) * SB], ps_k[:])
                ps_v = pqkv.tile([C, SB], F32, tag="q")
                for i in range(NEB):
                    nc.tensor.matmul(
                        ps_v[:], wv_sb[:, i, :], hts[i][:], start=(i == 0), stop=False
                    )
                nc.tensor.matmul(ps_v[:], bv_sb[:], ones_r[:], start=False, stop=True)
                vt_f = vtf.tile([C, SB], F32)
                nc.vector.tensor_copy(vt_f[:], ps_v[:])
                for tb in range(SB // TB):
                    ic = j * (SB // TB) + tb  # global t-chunk id
                    ps_t = pmix.tile([TB, TB], F32, tag="tr")
                    nc.tensor.transpose(ps_t[:], vt_f[:, tb * TB:(tb + 1) * TB], ident[:])
                    for h in range(HPC):
                        base = h * (D + 1)
                        nc.vector.tensor_copy(
                            v_sb[:, ic, base:base + D], ps_t[:, h * D:(h + 1) * D]
                        )
                        nc.vector.tensor_copy(
                            v_sb[:, ic, base + D:base + D + 1], ones_col_f[:]
                        )

                # ---- causal attention for s-block j (both heads) ----
                nchunks = (j + 1) * (SB // TB)
                for h in range(HPC):
                    hp = h * D
                    vb = h * (D + 1)
                    ps_ctx = pctx.tile([D + 1, SB], F32, tag="ctx")
                    for i in range(nchunks):
                        ps_sc = psc.tile([TB, SB], F32, tag="sc")
                        et = ep.tile([TB, SB], F32R, tag="et")
                        diag = i - j * (SB // TB)
                        # Columns f < 128*diag of a diagonal chunk are fully
                        # masked; skip them in scores/exp/mask/PV entirely.
                        off = TB * diag if diag > 0 else 0
                        w = SB - off
                        nc.tensor.matmul(
                            ps_sc[:, off:SB],
                            kT_sb[hp:hp + D, i * TB:(i + 1) * TB],
                            qT_sb[hp:hp + D, j * SB + off:(j + 1) * SB],
                            start=True, stop=True,
                        )
                        if diag >= 0:  # chunk straddling the causal boundary
                            et_f = ef.tile([TB, SB], F32, tag="etf")
                            nc.scalar.activation(
                                out=et_f[:, off:SB], in_=ps_sc[:, off:SB],
                                func=mybir.ActivationFunctionType.Exp, scale=float(SCALE),
                            )
                            nc.vector.tensor_mul(
                                et[:, off:SB], et_f[:, off:SB], mask0[:, 0:w]
                            )
                        else:
                            nc.scalar.activation(
                                out=et[:], in_=ps_sc[:],
                                func=mybir.ActivationFunctionType.Exp, scale=float(SCALE),
                            )
                        nc.tensor.matmul(
                            ps_ctx[:, off:SB],
                            v_sb[:, i, vb:vb + D + 1],
                            et[:, off:SB],
                            start=(i == 0), stop=(i == nchunks - 1),
                        )
                    # normalize: ctxT = ctx_hat / denom (denom = row D of ps_ctx)
                    ctx_f = ip.tile([D + 1, SB], F32, tag="ctxf")
                    nc.vector.tensor_copy(ctx_f[:], ps_ctx[:])
                    inv_f = ip.tile([1, SB], F32, tag="invf")
                    nc.vector.reciprocal(inv_f[:], ctx_f[D:D + 1, :])
                    inv_r = ip.tile([1, SB], F32R, tag="invr")
                    nc.vector.tensor_copy(inv_r[:], inv_f[:])
                    ps_in = pmix.tile([D, SB], F32, tag="inv")
                    nc.tensor.matmul(ps_in[:], ones64_r[:], inv_r[:], start=True, stop=True)
                    inv64 = ip.tile([D, SB], F32, tag="inv64")
                    nc.vector.tensor_copy(inv64[:], ps_in[:])
                    nc.vector.tensor_mul(
                        ctxT_sb[hp:hp + D, j * SB:(j + 1) * SB],
                        ctx_f[0:D, :],
                        inv64[:],
                    )

                # ---- partial out-projection for s-block j ----
                for tb in range(SB // TB):
                    sb = j * (SB // TB) + tb
                    for eh in range(E // SB):
                        ps_o = pmix.tile([TB, SB], F32, tag="y")
                        nc.tensor.matmul(
                            ps_o[:],
                            ctxT_sb[:, sb * TB:(sb + 1) * TB],
                            wo_sb[:, eh * SB:(eh + 1) * SB],
                            start=True, stop=True,
                        )
                        y_t = yp.tile([TB, SB], F32, tag="yt")
                        nc.vector.tensor_copy(y_t[:], ps_o[:])
                        nc.sync.dma_start(
                            out=y_part[sb * TB:(sb + 1) * TB, eh * SB:(eh + 1) * SB],
                            in_=y_t[:],
                        )

            # ---- sum partials across cores; each core keeps its row slice ----
            y_red = dram.tile([SB, E], F32)
            nc.gpsimd.collective_compute(
                "ReduceScatter",
                mybir.AluOpType.add,
                replica_groups=groups,
                ins=[y_part[:].opt()],
                outs=[y_red[:].opt()],
            )
            nc.sync.dma_start(out=ys[:], in_=y_red[:])

    nc.compile()
    return nc


def kernel(hidden_states, qkv_w, qkv_b, out_w, out_b):
    global _COMPILED, last_results
    if _COMPILED is None:
        _COMPILED = _build()
    nc = _COMPILED

    hT = np.ascontiguousarray(hidden_states.T.astype(np.float32))
    wr = qkv_w.astype(np.float32).reshape(E, H, 3, D)
    br = qkv_b.astype(np.float32).reshape(H, 3, D)
    wor = out_w.astype(np.float32).reshape(H, D, E)

    in_maps = []
    for c in range(N_CORES):
        heads = [HPC * c + h for h in range(HPC)]
        in_maps.append({
            "hTs": np.ascontiguousarray(hT[:, c * SB:(c + 1) * SB]),
            "wq": np.ascontiguousarray(wr[:, heads, 0, :].reshape(E, C)),
            "wk": np.ascontiguousarray(wr[:, heads, 1, :].reshape(E, C)),
            "wv": np.ascontiguousarray(wr[:, heads, 2, :].reshape(E, C)),
            "bq": np.ascontiguousarray(br[heads, 0, :].reshape(1, C)),
            "bk": np.ascontiguousarray(br[heads, 1, :].reshape(1, C)),
            "bv": np.ascontiguousarray(br[heads, 2, :].reshape(1, C)),
            "wo": np.ascontiguousarray(wor[heads].reshape(C, E)),
        })

    res = run_bass_kernel_spmd(nc, in_maps, list(range(N_CORES)))
    last_results = res
    out = np.concatenate([res.results[c]["ys"] for c in range(N_CORES)], axis=0)
    out = out + out_b.astype(np.float32)[None, :]
    return out.astype(np.float32)


# revision 12
# speedup vs baseline: 1.0871x; 1.0871x over previous
"""Trainium2 Bass kernel for 16-head causal MultiHeadAttention (S=4096, E=1024).

Sharding: tensor-parallel over heads across 8 NeuronCores, with
sequence-sharded I/O to minimize host<->device traffic (the wall-clock
bottleneck on tunneled cores):

- input: each core receives only its 512-column block of hT ([E, 512], fp16,
  1 MiB) plus its per-head weight slices (fp16); an on-device AllGather
  reconstructs the full hT in local DRAM.
- each core computes QKV projection for its 2 heads, flash-style causal
  attention in scoresT layout ([t, s_q], softmax denominator via a
  ones-column appended to V so no partition reductions are needed), and a
  partial out-projection over its 128 ctx channels into a DRAM scratch.
- output: an on-device ReduceScatter(add) sums the 8 partial [S, E] outputs
  in fp32 and hands each core its 512-row slice, cast to fp16 on the way out
  (1 MiB/core back to the host). The host concatenates slices and adds out_b.

QKV and out-proj matmuls run in fp16 (same PE rate as fp32r) with fp32 PSUM
accumulation; Q/K/V and attention internals stay float32r (TF32-like).
QKV projection, attention, and out-projection are fused into one loop over
512-row s-blocks (causality makes block j's attention depend only on K/V from
blocks <= j), so TensorE/ACT/DVE/DMA overlap across stages. The causal
boundary mask is generated on device with affine_select.
"""

import numpy as np

import concourse.bass as bass
import concourse.bacc as bacc
import concourse.mybir as mybir
from concourse.bass_utils import run_bass_kernel_spmd
from concourse.masks import make_identity
from concourse.tile import TileContext

N_CORES = 8
S = 4096
E = 1024
H = 16
D = 64
HPC = H // N_CORES          # heads per core = 2
C = HPC * D                 # ctx channels per core = 128
SCALE = 1.0 / np.sqrt(np.float32(E))  # note: sqrt(n_embd), per reference

SB = 512                    # s_q block (matmul free dim)
NSB = S // SB               # 8
TB = 128                    # t chunk (matmul contraction tile)
EB = 128                    # e chunk of the hidden dim
NEB = E // EB               # 8

F32 = mybir.dt.float32
F32R = mybir.dt.float32r
F16 = mybir.dt.float16

_COMPILED = None
last_results = None  # test harness reads exec_time_ns off this


def _build():
    nc = bacc.Bacc(None, target_bir_lowering=False, num_devices=N_CORES)

    hTs = nc.declare_dram_parameter("hTs", [E, SB], F16, isOutput=False)
    wq = nc.declare_dram_parameter("wq", [E, C], F16, isOutput=False)
    wk = nc.declare_dram_parameter("wk", [E, C], F16, isOutput=False)
    wv = nc.declare_dram_parameter("wv", [E, C], F16, isOutput=False)
    bq = nc.declare_dram_parameter("bq", [1, C], F16, isOutput=False)
    bk = nc.declare_dram_parameter("bk", [1, C], F16, isOutput=False)
    bv = nc.declare_dram_parameter("bv", [1, C], F16, isOutput=False)
    wo = nc.declare_dram_parameter("wo", [C, E], F16, isOutput=False)
    ys = nc.declare_dram_parameter("ys", [SB, E], F16, isOutput=True)

    groups = [list(range(N_CORES))]

    with TileContext(nc) as tc:
        with (
            tc.tile_pool(name="dram", bufs=1, space="DRAM") as dram,
            tc.tile_pool(name="singles", bufs=1) as singles,
            tc.tile_pool(name="big", bufs=1) as big,
            tc.tile_pool(name="htp", bufs=18) as htp,
            tc.tile_pool(name="vtf", bufs=3) as vtf,
            tc.tile_pool(name="ep", bufs=8) as ep,
            tc.tile_pool(name="ef", bufs=3) as ef,
            tc.tile_pool(name="ip", bufs=3) as ip,
            tc.tile_pool(name="pqkv", bufs=1, space="PSUM") as pqkv,
            tc.tile_pool(name="pmix", bufs=1, space="PSUM") as pmix,
            tc.tile_pool(name="psc", bufs=3, space="PSUM") as psc,
            tc.tile_pool(name="pctx", bufs=1, space="PSUM") as pctx,
            tc.tile_pool(name="yp", bufs=4) as yp,
        ):
            # ---- gather the sequence-sharded hidden states on device ----
            hTs_b = dram.tile([E, SB], F16)
            nc.sync.dma_start(out=hTs_b[:], in_=hTs[:])
            hT_g = dram.tile([NSB, E, SB], F16, addr_space="Shared")
            nc.gpsimd.collective_compute(
                "AllGather",
                mybir.AluOpType.bypass,
                replica_groups=groups,
                ins=[hTs_b[:].opt()],
                outs=[hT_g[:].opt()],
            )
            y_part = dram.tile([S, E], F32)   # this core's partial out-proj

            # Weights, biases, constants
            wq_sb = singles.tile([EB, NEB, C], F16)
            wk_sb = singles.tile([EB, NEB, C], F16)
            wv_sb = singles.tile([EB, NEB, C], F16)
            for w_dram, w_sb in ((wq, wq_sb), (wk, wk_sb), (wv, wv_sb)):
                nc.sync.dma_start(
                    out=w_sb[:], in_=w_dram.rearrange("(a p) m -> p a m", p=EB)
                )
            wo_sb = singles.tile([C, E], F16)
            nc.sync.dma_start(out=wo_sb[:], in_=wo[:])
            bq_sb = singles.tile([1, C], F16)
            bk_sb = singles.tile([1, C], F16)
            bv_sb = singles.tile([1, C], F16)
            nc.sync.dma_start(out=bq_sb[:], in_=bq[:])
            nc.sync.dma_start(out=bk_sb[:], in_=bk[:])
            nc.sync.dma_start(out=bv_sb[:], in_=bv[:])

            # causal step mask: mask0[p, f] = 1.0 where p <= f else 0.0
            mask0 = singles.tile([TB, SB], F32)
            nc.gpsimd.memset(mask0[:], 1.0)
            nc.gpsimd.affine_select(
                out=mask0[:], in_=mask0[:],
                compare_op=mybir.AluOpType.is_ge,
                fill=0.0,
                base=0,
                pattern=[[1, SB]],       # +1 per free element f
                channel_multiplier=-1,   # -1 per partition p => f - p >= 0
            )

            ones_f = singles.tile([1, SB], F32)
            nc.vector.memset(ones_f[:], 1.0)
            ones_h = singles.tile([1, SB], F16)
            nc.vector.tensor_copy(ones_h[:], ones_f[:])
            ones_col_f = singles.tile([TB, 1], F32)
            nc.vector.memset(ones_col_f[:], 1.0)
            ident = singles.tile([TB, TB], F32)
            make_identity(nc, ident[:])

            # Persistent activations
            qT_sb = big.tile([C, S], F32R)      # [c, s]
            kT_sb = big.tile([C, S], F32R)
            v_sb = big.tile([TB, S // TB, 2 * (D + 1)], F32R)
            ctxT_sb = big.tile([C, S], F16)

            ones64_f = singles.tile([1, D], F32)
            nc.vector.memset(ones64_f[:], 1.0)
            ones64_r = singles.tile([1, D], F32R)
            nc.vector.tensor_copy(ones64_r[:], ones64_f[:])

            for j in range(NSB):
                # ---- QKV projection for s-block j: three sequential
                # single-bank passes (q, k, v) over the held hT tiles ----
                hts = []
                for i in range(NEB):
                    ht = htp.tile([EB, SB], F16)
                    hts.append(ht)
                    nc.sync.dma_start(
                        out=ht[:], in_=hT_g[j, i * EB:(i + 1) * EB, :]
                    )
                ps_q = pqkv.tile([C, SB], F32, tag="q")
                for i in range(NEB):
                    nc.tensor.matmul(
                        ps_q[:], wq_sb[:, i, :], hts[i][:], start=(i == 0), stop=False
                    )
                nc.tensor.matmul(ps_q[:], bq_sb[:], ones_h[:], start=False, stop=True)
                nc.vector.tensor_copy(qT_sb[:, j * SB:(j + 1) * SB], ps_q[:])
                ps_k = pqkv.tile([C, SB], F32, tag="q")
                for i in range(NEB):
                    nc.tensor.matmul(
                        ps_k[:], wk_sb[:, i, :], hts[i][:], start=(i == 0), stop=False
                    )
                nc.tensor.matmul(ps_k[:], bk_sb[:], ones_h[:], start=False, stop=True)
                nc.vector.tensor_copy(kT_sb[:, j * SB:(j + 1) * SB], ps_k[:])
                ps_v = pqkv.tile([C, SB], F32, tag="q")
                for i in range(NEB):
                    nc.tensor.matmul(
                        ps_v[:], wv_sb[:, i, :], hts[i][:], start=(i == 0), stop=False
                    )
                nc.tensor.matmul(ps_v[:], bv_sb[:], ones_h[:], start=False, stop=True)
                vt_f = vtf.tile([C, SB], F32)
                nc.vector.tensor_copy(vt_f[:], ps_v[:])
                for tb in range(SB // TB):
                    ic = j * (SB // TB) + tb  # global t-chunk id
                    ps_t = pmix.tile([TB, TB], F32, tag="tr")
                    nc.tensor.transpose(ps_t[:], vt_f[:, tb * TB:(tb + 1) * TB], ident[:])
                    for h in range(HPC):
                        base = h * (D + 1)
                        nc.vector.tensor_copy(
                            v_sb[:, ic, base:base + D], ps_t[:, h * D:(h + 1) * D]
                        )
                        nc.vector.tensor_copy(
                            v_sb[:, ic, base + D:base + D + 1], ones_col_f[:]
                        )

                # ---- causal attention for s-block j (both heads) ----
                nchunks = (j + 1) * (SB // TB)
                for h in range(HPC):
                    hp = h * D
                    vb = h * (D + 1)
                    ps_ctx = pctx.tile([D + 1, SB], F32, tag="ctx")
                    for i in range(nchunks):
                        ps_sc = psc.tile([TB, SB], F32, tag="sc")
                        et = ep.tile([TB, SB], F32R, tag="et")
                        diag = i - j * (SB // TB)
                        # Columns f < 128*diag of a diagonal chunk are fully
                        # masked; skip them in scores/exp/mask/PV entirely.
                        off = TB * diag if diag > 0 else 0
                        w = SB - off
                        nc.tensor.matmul(
                            ps_sc[:, off:SB],
                            kT_sb[hp:hp + D, i * TB:(i + 1) * TB],
                            qT_sb[hp:hp + D, j * SB + off:(j + 1) * SB],
                            start=True, stop=True,
                        )
                        if diag >= 0:  # chunk straddling the causal boundary
                            et_f = ef.tile([TB, SB], F32, tag="etf")
                            nc.scalar.activation(
                                out=et_f[:, off:SB], in_=ps_sc[:, off:SB],
                                func=mybir.ActivationFunctionType.Exp, scale=float(SCALE),
                            )
                            nc.vector.tensor_mul(
                                et[:, off:SB], et_f[:, off:SB], mask0[:, 0:w]
                            )
                        else:
                            nc.scalar.activation(
                                out=et[:], in_=ps_sc[:],
                                func=mybir.ActivationFunctionType.Exp, scale=float(SCALE),
                            )
                        nc.tensor.matmul(
                            ps_ctx[:, off:SB],
                            v_sb[:, i, vb:vb + D + 1],
                            et[:, off:SB],
                            start=(i == 0), stop=(i == nchunks - 1),
                        )
                    # normalize: ctxT = ctx_hat / denom (denom = row D of ps_ctx)
                    ctx_f = ip.tile([D + 1, SB], F32, tag="ctxf")
                    nc.vector.tensor_copy(ctx_f[:], ps_ctx[:])
                    inv_f = ip.tile([1, SB], F32, tag="invf")
                    nc.vector.reciprocal(inv_f[:], ctx_f[D:D + 1, :])
                    inv_r = ip.tile([1, SB], F32R, tag="invr")
                    nc.vector.tensor_copy(inv_r[:], inv_f[:])
                    ps_in = pmix.tile([D, SB], F32, tag="inv")
                    nc.tensor.matmul(ps_in[:], ones64_r[:], inv_r[:], start=True, stop=True)
                    inv64 = ip.tile([D, SB], F32, tag="inv64")
                    nc.vector.tensor_copy(inv64[:], ps_in[:])
                    nc.vector.tensor_mul(
                        ctxT_sb[hp:hp + D, j * SB:(j + 1) * SB],
                        ctx_f[0:D, :],
                        inv64[:],
                    )

                # ---- partial out-projection for s-block j ----
                for tb in range(SB // TB):
                    sb = j * (SB // TB) + tb
                    for eh in range(E // SB):
                        ps_o = pmix.tile([TB, SB], F32, tag="y")
                        nc.tensor.matmul(
                            ps_o[:],
                            ctxT_sb[:, sb * TB:(sb + 1) * TB],
                            wo_sb[:, eh * SB:(eh + 1) * SB],
                            start=True, stop=True,
                        )
                        y_t = yp.tile([TB, SB], F32, tag="yt")
                        nc.vector.tensor_copy(y_t[:], ps_o[:])
                        nc.sync.dma_start(
                            out=y_part[sb * TB:(sb + 1) * TB, eh * SB:(eh + 1) * SB],
                            in_=y_t[:],
                        )

            # ---- sum partials across cores; each core keeps its row slice ----
            y_red = dram.tile([SB, E], F32)
            nc.gpsimd.collective_compute(
                "ReduceScatter",
                mybir.AluOpType.add,
                replica_groups=groups,
                ins=[y_part[:].opt()],
                outs=[y_red[:].opt()],
            )
            # fp32 -> fp16 cast on the way out (SWDGE casts during DMA)
            nc.gpsimd.dma_start(out=ys[:], in_=y_red[:])

    nc.compile()
    return nc


def kernel(hidden_states, qkv_w, qkv_b, out_w, out_b):
    global _COMPILED, last_results
    if _COMPILED is None:
        _COMPILED = _build()
    nc = _COMPILED

    hT = np.ascontiguousarray(hidden_states.T.astype(np.float16))
    wr = qkv_w.astype(np.float16).reshape(E, H, 3, D)
    br = qkv_b.astype(np.float16).reshape(H, 3, D)
    wor = out_w.astype(np.float16).reshape(H, D, E)

    in_maps = []
    for c in range(N_CORES):
        heads = [HPC * c + h for h in range(HPC)]
        in_maps.append({
            "hTs": np.ascontiguousarray(hT[:, c * SB:(c + 1) * SB]),
            "wq": np.ascontiguousarray(wr[:, heads, 0, :].reshape(E, C)),
            "wk": np.ascontiguousarray(wr[:, heads, 1, :].reshape(E, C)),
            "wv": np.ascontiguousarray(wr[:, heads, 2, :].reshape(E, C)),
            "bq": np.ascontiguousarray(br[heads, 0, :].reshape(1, C)),
            "bk": np.ascontiguousarray(br[heads, 1, :].reshape(1, C)),
            "bv": np.ascontiguousarray(br[heads, 2, :].reshape(1, C)),
            "wo": np.ascontiguousarray(wor[heads].reshape(C, E)),
        })

    res = run_bass_kernel_spmd(nc, in_maps, list(range(N_CORES)))
    last_results = res
    out = np.concatenate(
        [res.results[c]["ys"].astype(np.float32) for c in range(N_CORES)], axis=0
    )
    out = out + out_b.astype(np.float32)[None, :]
    return out.astype(np.float32)


# revision 20
# speedup vs baseline: 1.1887x; 1.0934x over previous
"""Trainium2 Bass kernel for 16-head causal MultiHeadAttention (S=4096, E=1024).

Sharding: tensor-parallel over heads across 8 NeuronCores, with
sequence-sharded I/O to minimize host<->device traffic (the wall-clock
bottleneck on tunneled cores):

- input: each core receives only its 512-column block of hT ([E, 512], fp16,
  1 MiB) plus its per-head weight slices (fp16); an on-device AllGather
  reconstructs the full hT in local DRAM.
- each core computes QKV projection for its 2 heads, flash-style causal
  attention in scoresT layout ([t, s_q], softmax denominator via a
  ones-column appended to V so no partition reductions are needed), and a
  partial out-projection over its 128 ctx channels into a DRAM scratch.
- output: an on-device ReduceScatter(add) sums the 8 partial [S, E] outputs
  in fp32 and hands each core its 512-row slice, cast to fp16 on the way out
  (1 MiB/core back to the host). The host concatenates slices and adds out_b.

QKV and out-proj matmuls run in fp16 (same PE rate as fp32r) with fp32 PSUM
accumulation; Q/K/V and attention internals stay float32r (TF32-like).
QKV projection, attention, and out-projection are fused into one loop over
512-row s-blocks (causality makes block j's attention depend only on K/V from
blocks <= j), so TensorE/ACT/DVE/DMA overlap across stages. The causal
boundary mask is generated on device with affine_select.
"""

import numpy as np

import jax

# Cache compiled XLA executables (the NEFF-wrapped custom call) on disk so
# repeat kernel() calls and fresh processes skip the ~0.7s walrus recompile.
try:
    jax.config.update("jax_compilation_cache_dir", "/tmp/jax_cache")
    jax.config.update("jax_persistent_cache_min_compile_time_secs", 0.0)
    jax.config.update("jax_persistent_cache_min_entry_size_bytes", -1)
except Exception:
    pass

import concourse.bass as bass
import concourse.bacc as bacc
import concourse.mybir as mybir
from concourse.bass_utils import run_bass_kernel_spmd
from concourse.masks import make_identity
from concourse.tile import TileContext

N_CORES = 8
S = 4096
E = 1024
H = 16
D = 64
HPC = H // N_CORES          # heads per core = 2
C = HPC * D                 # ctx channels per core = 128
SCALE = 1.0 / np.sqrt(np.float32(E))  # note: sqrt(n_embd), per reference

SB = 512                    # s_q block (matmul free dim)
NSB = S // SB               # 8
TB = 128                    # t chunk (matmul contraction tile)
EB = 128                    # e chunk of the hidden dim
NEB = E // EB               # 8

F32 = mybir.dt.float32
F32R = mybir.dt.float32r
F16 = mybir.dt.float16

_COMPILED = None
last_results = None  # test harness reads exec_time_ns off this


def _build():
    nc = bacc.Bacc(None, target_bir_lowering=False, num_devices=N_CORES)

    hTs = nc.declare_dram_parameter("hTs", [E, SB], F16, isOutput=False)
    wq = nc.declare_dram_parameter("wq", [E, C], F16, isOutput=False)
    wk = nc.declare_dram_parameter("wk", [E, C], F16, isOutput=False)
    wv = nc.declare_dram_parameter("wv", [E, C], F16, isOutput=False)
    bq = nc.declare_dram_parameter("bq", [1, C], F16, isOutput=False)
    bk = nc.declare_dram_parameter("bk", [1, C], F16, isOutput=False)
    bv = nc.declare_dram_parameter("bv", [1, C], F16, isOutput=False)
    wo = nc.declare_dram_parameter("wo", [C, E], F16, isOutput=False)
    ys = nc.declare_dram_parameter("ys", [SB, E], F16, isOutput=True)

    groups = [list(range(N_CORES))]

    with TileContext(nc) as tc:
        with (
            tc.tile_pool(name="dram", bufs=1, space="DRAM") as dram,
            tc.tile_pool(name="singles", bufs=1) as singles,
            tc.tile_pool(name="big", bufs=1) as big,
            tc.tile_pool(name="htp", bufs=18) as htp,
            tc.tile_pool(name="vtf", bufs=3) as vtf,
            tc.tile_pool(name="ep", bufs=8) as ep,
            tc.tile_pool(name="ef", bufs=3) as ef,
            tc.tile_pool(name="ip", bufs=3) as ip,
            tc.tile_pool(name="pqkv", bufs=1, space="PSUM") as pqkv,
            tc.tile_pool(name="pmix", bufs=1, space="PSUM") as pmix,
            tc.tile_pool(name="psc", bufs=3, space="PSUM") as psc,
            tc.tile_pool(name="pctx", bufs=1, space="PSUM") as pctx,
            tc.tile_pool(name="yp", bufs=4) as yp,
        ):
            # ---- gather the sequence-sharded hidden states on device ----
            hTs_b = dram.tile([E, SB], F16)
            nc.sync.dma_start(out=hTs_b[:], in_=hTs[:])
            hT_g = dram.tile([NSB, E, SB], F16, addr_space="Shared")
            nc.gpsimd.collective_compute(
                "AllGather",
                mybir.AluOpType.bypass,
                replica_groups=groups,
                ins=[hTs_b[:].opt()],
                outs=[hT_g[:].opt()],
            )
            y_part = dram.tile([S, E], F32)   # this core's partial out-proj
            # per-s-block ReduceScatter results: block j, rows of this core
            y_red = dram.tile([NSB, SB // N_CORES, E], F32)

            # Weights, biases, constants
            wq_sb = singles.tile([EB, NEB, C], F16)
            wk_sb = singles.tile([EB, NEB, C], F16)
            wv_sb = singles.tile([EB, NEB, C], F16)
            for w_dram, w_sb in ((wq, wq_sb), (wk, wk_sb), (wv, wv_sb)):
                nc.sync.dma_start(
                    out=w_sb[:], in_=w_dram.rearrange("(a p) m -> p a m", p=EB)
                )
            wo_sb = singles.tile([C, E], F16)
            nc.sync.dma_start(out=wo_sb[:], in_=wo[:])
            bq_sb = singles.tile([1, C], F16)
            bk_sb = singles.tile([1, C], F16)
            bv_sb = singles.tile([1, C], F16)
            nc.sync.dma_start(out=bq_sb[:], in_=bq[:])
            nc.sync.dma_start(out=bk_sb[:], in_=bk[:])
            nc.sync.dma_start(out=bv_sb[:], in_=bv[:])

            # causal step mask: mask0[p, f] = 1.0 where p <= f else 0.0
            mask0 = singles.tile([TB, SB], F32)
            nc.gpsimd.memset(mask0[:], 1.0)
            nc.gpsimd.affine_select(
                out=mask0[:], in_=mask0[:],
                compare_op=mybir.AluOpType.is_ge,
                fill=0.0,
                base=0,
                pattern=[[1, SB]],       # +1 per free element f
                channel_multiplier=-1,   # -1 per partition p => f - p >= 0
            )

            ones_f = singles.tile([1, SB], F32)
            nc.vector.memset(ones_f[:], 1.0)
            ones_h = singles.tile([1, SB], F16)
            nc.vector.tensor_copy(ones_h[:], ones_f[:])
            ident = singles.tile([TB, TB], F32)
            make_identity(nc, ident[:])

            # Persistent activations
            qT_sb = big.tile([C, S], F32R)      # [c, s]
            kT_sb = big.tile([C, S], F32R)
            v_sb = big.tile([TB, S // TB, 2 * (D + 1)], F32R)
            ctxT_sb = big.tile([C, S], F16)
            # ones-columns of the [V | 1] blocks never change: fill once
            ones_nt = singles.tile([TB, S // TB], F32)
            nc.vector.memset(ones_nt[:], 1.0)
            for h in range(HPC):
                col = h * (D + 1) + D
                nc.vector.tensor_copy(v_sb[:, :, col], ones_nt[:])

            ones64_f = singles.tile([1, D], F32)
            nc.vector.memset(ones64_f[:], 1.0)
            ones64_r = singles.tile([1, D], F32R)
            nc.vector.tensor_copy(ones64_r[:], ones64_f[:])

            for j in range(NSB):
                # ---- QKV projection for s-block j: three sequential
                # single-bank passes (q, k, v) over the held hT tiles ----
                hts = []
                for i in range(NEB):
                    ht = htp.tile([EB, SB], F16)
                    hts.append(ht)
                    nc.sync.dma_start(
                        out=ht[:], in_=hT_g[j, i * EB:(i + 1) * EB, :]
                    )
                ps_q = pqkv.tile([C, SB], F32, tag="q")
                for i in range(NEB):
                    nc.tensor.matmul(
                        ps_q[:], wq_sb[:, i, :], hts[i][:], start=(i == 0), stop=False
                    )
                nc.tensor.matmul(ps_q[:], bq_sb[:], ones_h[:], start=False, stop=True)
                nc.vector.tensor_copy(qT_sb[:, j * SB:(j + 1) * SB], ps_q[:])
                ps_k = pqkv.tile([C, SB], F32, tag="q")
                for i in range(NEB):
                    nc.tensor.matmul(
                        ps_k[:], wk_sb[:, i, :], hts[i][:], start=(i == 0), stop=False
                    )
                nc.tensor.matmul(ps_k[:], bk_sb[:], ones_h[:], start=False, stop=True)
                nc.vector.tensor_copy(kT_sb[:, j * SB:(j + 1) * SB], ps_k[:])
                ps_v = pqkv.tile([C, SB], F32, tag="q")
                for i in range(NEB):
                    nc.tensor.matmul(
                        ps_v[:], wv_sb[:, i, :], hts[i][:], start=(i == 0), stop=False
                    )
                nc.tensor.matmul(ps_v[:], bv_sb[:], ones_h[:], start=False, stop=True)
                vt_f = vtf.tile([C, SB], F32)
                nc.vector.tensor_copy(vt_f[:], ps_v[:])
                for tb in range(SB // TB):
                    ic = j * (SB // TB) + tb  # global t-chunk id
                    ps_t = pmix.tile([TB, TB], F32, tag="tr")
                    nc.tensor.transpose(ps_t[:], vt_f[:, tb * TB:(tb + 1) * TB], ident[:])
                    for h in range(HPC):
                        base = h * (D + 1)
                        nc.vector.tensor_copy(
                            v_sb[:, ic, base:base + D], ps_t[:, h * D:(h + 1) * D]
                        )

                # ---- causal attention for s-block j (both heads) ----
                nchunks = (j + 1) * (SB // TB)
                for h in range(HPC):
                    hp = h * D
                    vb = h * (D + 1)
                    ps_ctx = pctx.tile([D + 1, SB], F32, tag="ctx")
                    for i in range(nchunks):
                        ps_sc = psc.tile([TB, SB], F32, tag="sc")
                        et = ep.tile([TB, SB], F32R, tag="et")
                        diag = i - j * (SB // TB)
                        # Columns f < 128*diag of a diagonal chunk are fully
                        # masked; skip them in scores/exp/mask/PV entirely.
                        off = TB * diag if diag > 0 else 0
                        w = SB - off
                        nc.tensor.matmul(
                            ps_sc[:, off:SB],
                            kT_sb[hp:hp + D, i * TB:(i + 1) * TB],
                            qT_sb[hp:hp + D, j * SB + off:(j + 1) * SB],
                            start=True, stop=True,
                        )
                        if diag >= 0:  # chunk straddling the causal boundary
                            et_f = ef.tile([TB, SB], F32, tag="etf")
                            nc.scalar.activation(
                                out=et_f[:, off:SB], in_=ps_sc[:, off:SB],
                                func=mybir.ActivationFunctionType.Exp, scale=float(SCALE),
                            )
                            nc.vector.tensor_mul(
                                et[:, off:SB], et_f[:, off:SB], mask0[:, 0:w]
                            )
                        else:
                            nc.scalar.activation(
                                out=et[:], in_=ps_sc[:],
                                func=mybir.ActivationFunctionType.Exp, scale=float(SCALE),
                            )
                        nc.tensor.matmul(
                            ps_ctx[:, off:SB],
                            v_sb[:, i, vb:vb + D + 1],
                            et[:, off:SB],
                            start=(i == 0), stop=(i == nchunks - 1),
                        )
                    # normalize: ctxT = ctx_hat / denom (denom = row D of ps_ctx)
                    ctx_f = ip.tile([D + 1, SB], F32, tag="ctxf")
                    nc.vector.tensor_copy(ctx_f[:], ps_ctx[:])
                    inv_f = ip.tile([1, SB], F32, tag="invf")
                    nc.vector.reciprocal(inv_f[:], ctx_f[D:D + 1, :])
                    inv_r = ip.tile([1, SB], F32R, tag="invr")
                    nc.vector.tensor_copy(inv_r[:], inv_f[:])
                    ps_in = pmix.tile([D, SB], F32, tag="inv")
                    nc.tensor.matmul(ps_in[:], ones64_r[:], inv_r[:], start=True, stop=True)
                    inv64 = ip.tile([D, SB], F32, tag="inv64")
                    nc.vector.tensor_copy(inv64[:], ps_in[:])
                    nc.vector.tensor_mul(
                        ctxT_sb[hp:hp + D, j * SB:(j + 1) * SB],
                        ctx_f[0:D, :],
                        inv64[:],
                    )

                # ---- partial out-projection for s-block j ----
                for tb in range(SB // TB):
                    sb = j * (SB // TB) + tb
                    for eh in range(E // SB):
                        ps_o = pmix.tile([TB, SB], F32, tag="y")
                        nc.tensor.matmul(
                            ps_o[:],
                            ctxT_sb[:, sb * TB:(sb + 1) * TB],
                            wo_sb[:, eh * SB:(eh + 1) * SB],
                            start=True, stop=True,
                        )
                        y_t = yp.tile([TB, SB], F32, tag="yt")
                        nc.vector.tensor_copy(y_t[:], ps_o[:])
                        nc.sync.dma_start(
                            out=y_part[sb * TB:(sb + 1) * TB, eh * SB:(eh + 1) * SB],
                            in_=y_t[:],
                        )

                # ---- sum this s-block's partials across cores now, so the
                # collective overlaps with the next block's compute; each
                # core keeps 64 rows of the summed block ----
                nc.gpsimd.collective_compute(
                    "ReduceScatter",
                    mybir.AluOpType.add,
                    replica_groups=groups,
                    ins=[y_part[j * SB:(j + 1) * SB, :].opt()],
                    outs=[y_red[j].opt()],
                )

            # fp32 -> fp16 cast on the way out (SWDGE casts during DMA)
            nc.gpsimd.dma_start(
                out=ys.rearrange("(a p) m -> a p m", a=NSB),
                in_=y_red[:],
            )

    nc.compile()
    return nc


def kernel(hidden_states, qkv_w, qkv_b, out_w, out_b):
    global _COMPILED, last_results
    if _COMPILED is None:
        _COMPILED = _build()
    nc = _COMPILED

    hT = np.ascontiguousarray(hidden_states.T.astype(np.float16))
    wr = qkv_w.astype(np.float16).reshape(E, H, 3, D)
    br = qkv_b.astype(np.float16).reshape(H, 3, D)
    wor = out_w.astype(np.float16).reshape(H, D, E)

    in_maps = []
    for c in range(N_CORES):
        heads = [HPC * c + h for h in range(HPC)]
        in_maps.append({
            "hTs": np.ascontiguousarray(hT[:, c * SB:(c + 1) * SB]),
            "wq": np.ascontiguousarray(wr[:, heads, 0, :].reshape(E, C)),
            "wk": np.ascontiguousarray(wr[:, heads, 1, :].reshape(E, C)),
            "wv": np.ascontiguousarray(wr[:, heads, 2, :].reshape(E, C)),
            "bq": np.ascontiguousarray(br[heads, 0, :].reshape(1, C)),
            "bk": np.ascontiguousarray(br[heads, 1, :].reshape(1, C)),
            "bv": np.ascontiguousarray(br[heads, 2, :].reshape(1, C)),
            "wo": np.ascontiguousarray(wor[heads].reshape(C, E)),
        })

    res = run_bass_kernel_spmd(nc, in_maps, list(range(N_CORES)))
    last_results = res
    # core c's ys holds, for each s-block j, rows [j*SB + c*64, j*SB + (c+1)*64)
    rows_per = SB // N_CORES
    out = np.empty((S, E), dtype=np.float32)
    for c in range(N_CORES):
        frag = res.results[c]["ys"].reshape(NSB, rows_per, E)
        for j in range(NSB):
            r0 = j * SB + c * rows_per
            out[r0:r0 + rows_per] = frag[j]
    out += out_b.astype(np.float32)[None, :]
    return out
